# revision 24
# baseline (speedup 1.0000x reference)
"""Multi-head attention (B=4, N=1024, C=1024, H=16, D=64) on 8 Trainium2 cores.

Sharding: batch x head-half tensor parallel, no collectives. Core i handles
batch b = i//2 and heads (i%2)*8..+8 for ALL 1024 queries of that batch: it
projects q/k/v for its 8 heads only (no duplicated k/v work between the two
cores of a batch), runs attention, and computes the PARTIAL output projection
y_i = o_i @ w_out[rows of its 512 e-dims]. The host sums each batch's two
partials and adds the bias -- the output projection is linear in the head
dimension, so the pair-sum equals the full projection.

Matmuls run in fp16 (1 PE cycle/row, FWL weight loads). Accumulation is fp32
in PSUM. exp is computed as exp(s/8 - 12*ln2) so unnormalized attention
outputs stay in fp16 range; the 2^-12 factor cancels in the softmax
normalization. The softmax denominator rides along as a ones-column in v
(key mask folded into both), its reciprocal is computed with the fast DVE
approximation (~18 bits) and partition-broadcast on the otherwise idle GpSimd
engine, keeping the PE free of normalization work. v tiles are padded to 128
weight columns (cols 65..127 zero) so the AV matmuls get fast weight loads.

Per-core pipeline (x^T and packed weights are prepared on the host):
  1. Per head pair t (4 pairs): k^T/q^T column projections (K=co tiles),
     then S^T = k^T.T @ q^T per key m-tile, the pair alternating PE row
     groups 0/64 so its two matmuls overlap -> exp on ACT -> E.
  2. v = x @ w_v -> [m, 8 heads, d + ones column], mask folded in.
  3. Per head: out^T (unnormalized) + denominator via the ones column ->
     stage to SBUF; per head quad: fast reciprocal, gpsimd broadcast,
     DVE normalize into o^T.
  4. y_partial = o^T.T @ w_out (K=4 e-tiles), fp16, DMA out.
"""

import os

import numpy as np

import concourse.bacc as bacc
import concourse.mybir as mybir
import concourse.tile as tile
from concourse.bass_utils import run_bass_kernel_spmd

F32 = mybir.dt.float32
F16 = mybir.dt.float16

B, N, C = 4, 1024, 1024
H, D = 16, 64
P = 128
CO = C // P       # 8 contraction tiles
MO = N // P       # 8 key m-tiles
NO = N // P       # 8 output row tiles
HL = 8            # heads per core
T = HL // 2       # 4 head pairs per core
EO = T            # 4 e-tiles (one per pair) for the output projection
NH = N // 2       # 512-column matmul streams (PSUM bank)
ATT_SCALE = D ** -0.5
EXP_BIAS = float(-12.0 * np.log(2.0))  # keep out^T in fp16 range
N_CORES = 8


def build_nc():
    nc = bacc.Bacc()
    xbT = nc.declare_dram_parameter("xbT", [C, N], F16, isOutput=False)
    maskb = nc.declare_dram_parameter("maskb", [N], F32, isOutput=False)
    wq_pk = nc.declare_dram_parameter("wq_pk", [T, P, CO, P], F16,
                                      isOutput=False)
    wk_pk = nc.declare_dram_parameter("wk_pk", [T, P, CO, P], F16,
                                      isOutput=False)
    wv_pk = nc.declare_dram_parameter("wv_pk", [P, CO, HL * D], F16,
                                      isOutput=False)
    wo_pk = nc.declare_dram_parameter("wo_pk", [P, EO, C], F16, isOutput=False)
    y = nc.declare_dram_parameter("y", [N, C], F16, isOutput=True)

    xbT_t = xbT.rearrange("(co p) m -> p co m", p=P)
    y_t = y.rearrange("(no p) c -> p no c", p=P)

    with tile.TileContext(nc) as tc:
        with tc.tile_pool(name="consts", bufs=1) as consts, \
             tc.tile_pool(name="persist", bufs=1) as persist:
            # ---- constants ----
            onesH = consts.tile([P, HL], F16)
            nc.vector.memset(onesH[:], 1.0)
            ones_q = consts.tile([97, 64], F32)
            nc.vector.memset(ones_q[:], 1.0)
            mask_sb = consts.tile([P, MO], F32)
            ebias = consts.tile([P, 1], F32)
            nc.vector.memset(ebias[:], EXP_BIAS)

            # ---- persistent tensors ----
            qT = persist.tile([P, T, N], F16)            # q^T: [e, n]
            kT = persist.tile([P, T, N], F16)            # k^T: [e, m]
            v_sb = persist.tile([P, MO, HL, P], F16)     # v + ones col + pad
            o_sb = persist.tile([P, EO, N], F16)         # o^T: [e, n]
            wo = persist.tile([P, EO, C], F16)           # w_out staged late
            # zero the AV weight pad (cols 65..127) once
            nc.vector.memset(v_sb[:, :, :, D + 1:], 0.0)

            with tc.tile_pool(name="xT_pool", bufs=1) as xT_pool, \
                 tc.tile_pool(name="w_pool", bufs=4) as w_pool, \
                 tc.tile_pool(name="wv_pool", bufs=1) as wv_pool, \
                 tc.tile_pool(name="E_pool", bufs=2) as E_pool, \
                 tc.tile_pool(name="ou_pool", bufs=6) as ou_pool, \
                 tc.tile_pool(name="sm_pool", bufs=3) as sm_pool, \
                 tc.tile_pool(name="ysb_pool", bufs=6) as ysb_pool, \
                 tc.tile_pool(name="ps_proj", bufs=2, space="PSUM") as ps_proj, \
                 tc.tile_pool(name="ps_s", bufs=2, space="PSUM") as ps_s, \
                 tc.tile_pool(name="ps_av", bufs=2, space="PSUM") as ps_av:
                xT = xT_pool.tile([P, CO, N], F16)       # x^T: [c, m]

                # ---- first-pair weights + x, finely split so the first
                # k-projection pass isn't gated on per-queue DMA bandwidth
                wk0 = w_pool.tile([P, CO, P], F16, tag="wqk")
                wq0 = w_pool.tile([P, CO, P], F16, tag="wqk")
                # descriptor generation runs on BOTH hwdge sequencers: the
                # scalar engine issues the x pieces while sync issues weights
                for co in range(CO):
                    nc.sync.dma_start(wk0[:, co, :], wk_pk[0, :, co, :])
                    nc.scalar.dma_start(xT[:, co, 0:NH],
                                        xbT_t[:, co, 0:NH])
                for co in range(CO):
                    nc.sync.dma_start(wq0[:, co, :], wq_pk[0, :, co, :])
                    nc.scalar.dma_start(xT[:, co, NH:N],
                                        xbT_t[:, co, NH:N])
                nc.sync.dma_start(mask_sb[:],
                                  maskb.rearrange("(o p) -> p o", p=P))

                def kq_proj(t, wk=None, wq=None):
                    if wk is None:
                        wk = w_pool.tile([P, CO, P], F16, tag="wqk",
                                         name=f"wk{t}")
                        nc.sync.dma_start(wk[:], wk_pk[t])
                    if wq is None:
                        wq = w_pool.tile([P, CO, P], F16, tag="wqk",
                                         name=f"wq{t}")
                        nc.sync.dma_start(wq[:], wq_pk[t])
                    for half in range(2):
                        pk = ps_proj.tile([P, NH], F32, tag="pp",
                                          name=f"pk{t}_{half}")
                        for co in range(CO):
                            nc.tensor.matmul(
                                pk[:], wk[:, co, :],
                                xT[:, co, half * NH:(half + 1) * NH],
                                start=(co == 0), stop=(co == CO - 1))
                        nc.vector.tensor_copy(
                            kT[:, t, half * NH:(half + 1) * NH], pk[:])
                    for half in range(2):
                        pq = ps_proj.tile([P, NH], F32, tag="pp",
                                          name=f"pq{t}_{half}")
                        for co in range(CO):
                            nc.tensor.matmul(
                                pq[:], wq[:, co, :],
                                xT[:, co, half * NH:(half + 1) * NH],
                                start=(co == 0), stop=(co == CO - 1))
                        nc.vector.tensor_copy(
                            qT[:, t, half * NH:(half + 1) * NH], pq[:])

                def s_exp(t):
                    # S^T + exp: one [128, 1024] psum per (mo, q-half) holds
                    # both heads of the pair (the back-to-back matmuls
                    # alternate PE row groups 0/64 and overlap on the array);
                    # one strided EXP covers both heads
                    E_t = E_pool.tile([P, MO, 2, N], F16, tag="E",
                                      name=f"E{t}")
                    for mo in range(MO):
                        for qh in range(2):
                            pss = ps_s.tile([P, 2 * NH], F32, tag="ps_s",
                                            name=f"pss{t}_{mo}_{qh}")
                            for j in range(2):
                                pb = 64 * j
                                nc.tensor.matmul(
                                    pss[:, j * NH:(j + 1) * NH],
                                    kT[pb:pb + 64, t, mo * P:(mo + 1) * P],
                                    qT[pb:pb + 64, t, qh * NH:(qh + 1) * NH],
                                    start=True, stop=True)
                            nc.scalar.activation(
                                E_t[:, mo, :, qh * NH:(qh + 1) * NH],
                                pss[:].rearrange("p (j n) -> p j n", j=2),
                                mybir.ActivationFunctionType.Exp,
                                bias=ebias[:], scale=ATT_SCALE)
                    return E_t

                def v_proj():
                    wv = wv_pool.tile([P, CO, HL * D], F16, tag="wv")
                    nc.sync.dma_start(wv[:], wv_pk[:])
                    for mo in range(MO):
                        pv = ps_proj.tile([P, NH], F32, tag="pp",
                                          name=f"pv{mo}")
                        for co in range(CO):
                            nc.tensor.matmul(
                                pv[:], xT[:, co, mo * P:(mo + 1) * P],
                                wv[:, co, :],
                                start=(co == 0), stop=(co == CO - 1))
                        nc.vector.tensor_scalar_mul(
                            v_sb[:, mo, :, 0:D],
                            pv[:].rearrange("p (h d) -> p h d", d=D),
                            mask_sb[:, mo:mo + 1])
                    for mo in range(MO):
                        nc.vector.tensor_scalar_mul(
                            v_sb[:, mo, :, D], onesH[:], mask_sb[:, mo:mo + 1])

                o_un = {}
                norm_state = {}

                def norm_qh(quad, qh):
                    # normalize the quad's q-half: fast 1/den on DVE,
                    # ones-row PE broadcast, multiply into o^T
                    den_q = norm_state[quad]
                    if qh == 0:
                        norm_state["rcp"] = sm_pool.tile(
                            [97, N], F32, tag="sm", name=f"rcp{quad}")
                    rcp_q = norm_state["rcp"]
                    sl = slice(qh * NH, (qh + 1) * NH)
                    nc.vector.reciprocal_approx_fast(
                        rcp_q[:, sl], den_q[:, sl])
                    for r in range(4):
                        h = 4 * quad + r
                        t, pb = h // 2, 64 * (h % 2)
                        pbc = ps_av.tile([P, NH], F32, tag="ps_av",
                                         name=f"pbc{h}_{qh}")
                        nc.tensor.matmul(
                            pbc[0:64, :], ones_q[32 * r:32 * r + 1, :],
                            rcp_q[32 * r:32 * r + 1, sl],
                            start=True, stop=True,
                            tile_position=(32 * r, 0))
                        nc.vector.tensor_mul(
                            o_sb[pb:pb + 64, t, sl],
                            o_un[h][:, sl], pbc[0:64, :])
                        if qh == 1:
                            o_un.pop(h)

                def av(t, E_t, mid=None):
                    # unnormalized out^T + denominator via the ones column
                    # (quad's head r at partition 32r: engine partition
                    # offsets must be multiples of 32)
                    if t % 2 == 0:
                        den_q = sm_pool.tile([97, N], F32, tag="sm",
                                             name=f"den{t // 2}")
                        nc.vector.memset(den_q[:], 1.0)
                        norm_state[t // 2] = den_q
                    den_q = norm_state[t // 2]
                    for j in range(2):
                        o_un[2 * t + j] = ou_pool.tile(
                            [64, N], F16, tag="ou", name=f"ou{2 * t + j}")
                    for qh in range(2):
                        for j in range(2):
                            h = 2 * t + j
                            r = h % 4
                            pav = ps_av.tile([P, NH], F32, tag="ps_av",
                                             name=f"pav{h}_{qh}")
                            for mo in range(MO):
                                nc.tensor.matmul(
                                    pav[:], v_sb[:, mo, h, :],
                                    E_t[:, mo, j, qh * NH:(qh + 1) * NH],
                                    start=(mo == 0), stop=(mo == MO - 1))
                            nc.vector.tensor_copy(
                                o_un[h][:, qh * NH:(qh + 1) * NH],
                                pav[0:D, :])
                            nc.vector.tensor_copy(
                                den_q[32 * r:32 * r + 1,
                                      qh * NH:(qh + 1) * NH],
                                pav[D:D + 1, :])
                        if qh == 0 and mid is not None:
                            mid()

                # skewed pipeline: attention of pair t overlaps k/q
                # projections of pair t+1 and S/exp of pair t+1.
                kq_proj(0, wk=wk0, wq=wq0)
                Es = {0: s_exp(0)}
                kq_proj(1)
                Es[1] = s_exp(1)
                kq_proj(2)
                v_proj()
                # norm chains are emitted with a lag so their PE broadcasts
                # never wait on the DVE reciprocal/cast pipeline
                for t in range(T):
                    if t + 2 < T:
                        Es[t + 2] = s_exp(t + 2)
                    if t + 3 < T:
                        kq_proj(t + 3)
                    if t == 1:  # stage w_out late, off the critical DMA path
                        for eo in range(EO):
                            nc.sync.dma_start(wo[:, eo, :], wo_pk[:, eo, :])
                    if t == 2:
                        norm_qh(0, 0)
                    av(t, Es.pop(t),
                       mid=(lambda: norm_qh(0, 1)) if t == 2 else None)
                norm_qh(1, 0)
                norm_qh(1, 1)

                # ---- partial output projection (host adds bias) ----
                for no in range(NO):
                    for ch in range(2):
                        pool = ps_proj if (no * 2 + ch) % 2 == 0 else ps_av
                        py = pool.tile([P, NH], F32,
                                       tag="pp" if pool is ps_proj
                                       else "ps_av",
                                       name=f"py{no}_{ch}")
                        for eo in range(EO):
                            nc.tensor.matmul(
                                py[:], o_sb[:, eo, no * P:(no + 1) * P],
                                wo[:, eo, ch * NH:(ch + 1) * NH],
                                start=(eo == 0), stop=(eo == EO - 1))
                        ysb = ysb_pool.tile([P, NH], F16, tag="ysb",
                                            name=f"ysb{no}_{ch}")
                        if no == NO - 1 and ch == 1:
                            nc.vector.tensor_copy(ysb[:], py[:])
                            nc.sync.dma_start(
                                y_t[:, no, ch * NH:ch * NH + 256],
                                ysb[:, 0:256])
                            nc.scalar.dma_start(
                                y_t[:, no, ch * NH + 256:(ch + 1) * NH],
                                ysb[:, 256:NH])
                        else:
                            nc.scalar.copy(ysb[:], py[:])
                            nc.sync.dma_start(
                                y_t[:, no, ch * NH:(ch + 1) * NH], ysb[:])

    nc.finalize()
    return nc


_NC_CACHE = None


def _get_nc():
    global _NC_CACHE
    if _NC_CACHE is None:
        _NC_CACHE = build_nc()
    return _NC_CACHE


def _make_in_maps(x, mask, w_qkv, w_out, b_out):
    x = np.asarray(x, dtype=np.float32)
    mask_f = np.asarray(mask).astype(np.float32)
    wqkv_h = np.asarray(w_qkv).astype(np.float16)
    wout_h = np.asarray(w_out).astype(np.float16)
    # w_qkv [C, 3HD]: q cols 0:C, k cols C:2C, v cols 2C:3C; head h at h*D
    wq4 = wqkv_h.reshape(CO, P, 3 * H * D)
    in_maps = []
    for i in range(N_CORES):
        b, hh = i // 2, i % 2
        e0 = hh * HL * D  # first e-col of this core's head block
        xbT = np.ascontiguousarray(x[b].T.astype(np.float16))
        wq_pk = np.ascontiguousarray(
            wq4[:, :, e0:e0 + HL * D].reshape(CO, P, T, P)
            .transpose(2, 1, 0, 3))
        wk_pk = np.ascontiguousarray(
            wq4[:, :, C + e0:C + e0 + HL * D].reshape(CO, P, T, P)
            .transpose(2, 1, 0, 3))
        wv_pk = np.ascontiguousarray(
            wq4[:, :, 2 * C + e0:2 * C + e0 + HL * D].transpose(1, 0, 2))
        wo_pk = np.ascontiguousarray(
            wout_h[e0:e0 + HL * D, :].reshape(EO, P, C).transpose(1, 0, 2))
        in_maps.append({"xbT": xbT, "maskb": mask_f[b], "wq_pk": wq_pk,
                        "wk_pk": wk_pk, "wv_pk": wv_pk, "wo_pk": wo_pk})
    return in_maps


def run_kernel(x, mask, w_qkv, w_out, b_out, trace=False):
    """Run on 8 cores; returns (full output [B,N,C], BassKernelResults)."""
    nc = _get_nc()
    in_maps = _make_in_maps(x, mask, w_qkv, w_out, b_out)
    res = run_bass_kernel_spmd(nc, in_maps, core_ids=list(range(N_CORES)),
                               trace=trace)
    bias = np.asarray(b_out, dtype=np.float32)
    out = np.empty((B, N, C), dtype=np.float32)
    for b in range(B):
        out[b] = (res.results[2 * b]["y"].astype(np.float32)
                  + res.results[2 * b + 1]["y"].astype(np.float32) + bias)
    return out, res


def kernel(x, mask, w_qkv, w_out, b_out):
    os.environ.setdefault("BASS_NEVER_TRACE", "1")
    out, _ = run_kernel(x, mask, w_qkv, w_out, b_out, trace=False)
    return out


# revision 25
# speedup vs baseline: 1.1116x; 1.1116x over previous
"""Multi-head attention (B=4, N=1024, C=1024, H=16, D=64) on 8 Trainium2 cores.

Sharding: batch x head-half tensor parallel, no collectives. Core i handles
batch b = i//2 and heads (i%2)*8..+8 for ALL 1024 queries of that batch: it
projects q/k/v for its 8 heads only (no duplicated k/v work between the two
cores of a batch), runs attention, and computes the PARTIAL output projection
y_i = o_i @ w_out[rows of its 512 e-dims]. The host sums each batch's two
partials and adds the bias -- the output projection is linear in the head
dimension, so the pair-sum equals the full projection.

Matmuls run in fp16 (1 PE cycle/row, FWL weight loads). Accumulation is fp32
in PSUM. exp is computed as exp(s/8 - 12*ln2) so unnormalized attention
outputs stay in fp16 range; the 2^-12 factor cancels in the softmax
normalization. The softmax denominator rides along as a ones-column in v
(key mask folded into both), its reciprocal is computed with the fast DVE
approximation (~18 bits) and partition-broadcast on the otherwise idle GpSimd
engine, keeping the PE free of normalization work. v tiles are padded to 128
weight columns (cols 65..127 zero) so the AV matmuls get fast weight loads.

Per-core pipeline (x^T and packed weights are prepared on the host):
  1. Per head pair t (4 pairs): k^T/q^T column projections (K=co tiles),
     then S^T = k^T.T @ q^T per key m-tile, the pair alternating PE row
     groups 0/64 so its two matmuls overlap -> exp on ACT -> E.
  2. v = x @ w_v -> [m, 8 heads, d + ones column], mask folded in.
  3. Per head: out^T (unnormalized) + denominator via the ones column ->
     stage to SBUF; per head quad: fast reciprocal, gpsimd broadcast,
     DVE normalize into o^T.
  4. y_partial = o^T.T @ w_out (K=4 e-tiles), fp16, DMA out.
"""

import os

import numpy as np

import concourse.bacc as bacc
import concourse.mybir as mybir
import concourse.tile as tile
from concourse.bass_utils import run_bass_kernel_spmd

F32 = mybir.dt.float32
F16 = mybir.dt.float16

B, N, C = 4, 1024, 1024
H, D = 16, 64
P = 128
CO = C // P       # 8 contraction tiles
MO = N // P       # 8 key m-tiles
NO = N // P       # 8 output row tiles
HL = 8            # heads per core
T = HL // 2       # 4 head pairs per core
EO = T            # 4 e-tiles (one per pair) for the output projection
NH = N // 2       # 512-column matmul streams (PSUM bank)
ATT_SCALE = D ** -0.5
EXP_BIAS = float(-12.0 * np.log(2.0))  # keep out^T in fp16 range
N_CORES = 8


def build_nc():
    nc = bacc.Bacc()
    xbT = nc.declare_dram_parameter("xbT", [C, N], F16, isOutput=False)
    maskb = nc.declare_dram_parameter("maskb", [N], F32, isOutput=False)
    wq_pk = nc.declare_dram_parameter("wq_pk", [T, P, CO, P], F16,
                                      isOutput=False)
    wk_pk = nc.declare_dram_parameter("wk_pk", [T, P, CO, P], F16,
                                      isOutput=False)
    wv_pk = nc.declare_dram_parameter("wv_pk", [P, CO, HL * D], F16,
                                      isOutput=False)
    wo_pk = nc.declare_dram_parameter("wo_pk", [P, EO, C], F16, isOutput=False)
    y = nc.declare_dram_parameter("y", [N, C], F16, isOutput=True)

    xbT_t = xbT.rearrange("(co p) m -> p co m", p=P)
    y_t = y.rearrange("(no p) c -> p no c", p=P)

    with tile.TileContext(nc) as tc:
        with tc.tile_pool(name="consts", bufs=1) as consts, \
             tc.tile_pool(name="persist", bufs=1) as persist:
            # ---- constants ----
            onesH = consts.tile([P, HL], F16)
            nc.vector.memset(onesH[:], 1.0)
            ones_q = consts.tile([97, 64], F16)
            nc.vector.memset(ones_q[:], 1.0)
            mask_sb = consts.tile([P, MO], F32)
            ebias = consts.tile([P, 1], F32)
            nc.vector.memset(ebias[:], EXP_BIAS)

            # ---- persistent tensors ----
            qT = persist.tile([P, T, N], F16)            # q^T: [e, n]
            kT = persist.tile([P, T, N], F16)            # k^T: [e, m]
            v_sb = persist.tile([P, MO, HL, P], F16)     # v + ones col + pad
            o_sb = persist.tile([P, EO, N], F16)         # o^T: [e, n]
            wo = persist.tile([P, EO, C], F16)           # w_out staged late
            # zero the AV weight pad (cols 65..127) once
            nc.vector.memset(v_sb[:, :, :, D + 1:], 0.0)

            with tc.tile_pool(name="xT_pool", bufs=1) as xT_pool, \
                 tc.tile_pool(name="w_pool", bufs=4) as w_pool, \
                 tc.tile_pool(name="wv_pool", bufs=1) as wv_pool, \
                 tc.tile_pool(name="E_pool", bufs=2) as E_pool, \
                 tc.tile_pool(name="ou_pool", bufs=6) as ou_pool, \
                 tc.tile_pool(name="sm_pool", bufs=3) as sm_pool, \
                 tc.tile_pool(name="rb_pool", bufs=2) as rb_pool, \
                 tc.tile_pool(name="ysb_pool", bufs=6) as ysb_pool, \
                 tc.tile_pool(name="ps_proj", bufs=2, space="PSUM") as ps_proj, \
                 tc.tile_pool(name="ps_s", bufs=2, space="PSUM") as ps_s, \
                 tc.tile_pool(name="ps_av", bufs=2, space="PSUM") as ps_av:
                xT = xT_pool.tile([P, CO, N], F16)       # x^T: [c, m]

                # ---- first-pair weights + x, finely split so the first
                # k-projection pass isn't gated on per-queue DMA bandwidth
                wk0 = w_pool.tile([P, CO, P], F16, tag="wqk")
                wq0 = w_pool.tile([P, CO, P], F16, tag="wqk")
                # descriptor generation runs on BOTH hwdge sequencers: the
                # scalar engine issues the x pieces while sync issues weights
                for co in range(CO):
                    nc.sync.dma_start(wk0[:, co, :], wk_pk[0, :, co, :])
                    nc.scalar.dma_start(xT[:, co, 0:NH],
                                        xbT_t[:, co, 0:NH])
                for co in range(CO):
                    nc.sync.dma_start(wq0[:, co, :], wq_pk[0, :, co, :])
                    nc.scalar.dma_start(xT[:, co, NH:N],
                                        xbT_t[:, co, NH:N])
                nc.sync.dma_start(mask_sb[:],
                                  maskb.rearrange("(o p) -> p o", p=P))

                def kq_proj(t, wk=None, wq=None):
                    if wk is None:
                        wk = w_pool.tile([P, CO, P], F16, tag="wqk",
                                         name=f"wk{t}")
                        nc.sync.dma_start(wk[:], wk_pk[t])
                    if wq is None:
                        wq = w_pool.tile([P, CO, P], F16, tag="wqk",
                                         name=f"wq{t}")
                        nc.sync.dma_start(wq[:], wq_pk[t])
                    for half in range(2):
                        pk = ps_proj.tile([P, NH], F32, tag="pp",
                                          name=f"pk{t}_{half}")
                        for co in range(CO):
                            nc.tensor.matmul(
                                pk[:], wk[:, co, :],
                                xT[:, co, half * NH:(half + 1) * NH],
                                start=(co == 0), stop=(co == CO - 1))
                        nc.vector.tensor_copy(
                            kT[:, t, half * NH:(half + 1) * NH], pk[:])
                    for half in range(2):
                        pq = ps_proj.tile([P, NH], F32, tag="pp",
                                          name=f"pq{t}_{half}")
                        for co in range(CO):
                            nc.tensor.matmul(
                                pq[:], wq[:, co, :],
                                xT[:, co, half * NH:(half + 1) * NH],
                                start=(co == 0), stop=(co == CO - 1))
                        nc.vector.tensor_copy(
                            qT[:, t, half * NH:(half + 1) * NH], pq[:])

                def s_exp(t):
                    # S^T + exp: one [128, 1024] psum per (mo, q-half) holds
                    # both heads of the pair (the back-to-back matmuls
                    # alternate PE row groups 0/64 and overlap on the array);
                    # one strided EXP covers both heads
                    E_t = E_pool.tile([P, MO, 2, N], F16, tag="E",
                                      name=f"E{t}")
                    for mo in range(MO):
                        for qh in range(2):
                            pss = ps_s.tile([P, 2 * NH], F32, tag="ps_s",
                                            name=f"pss{t}_{mo}_{qh}")
                            for j in range(2):
                                pb = 64 * j
                                nc.tensor.matmul(
                                    pss[:, j * NH:(j + 1) * NH],
                                    kT[pb:pb + 64, t, mo * P:(mo + 1) * P],
                                    qT[pb:pb + 64, t, qh * NH:(qh + 1) * NH],
                                    start=True, stop=True)
                            nc.scalar.activation(
                                E_t[:, mo, :, qh * NH:(qh + 1) * NH],
                                pss[:].rearrange("p (j n) -> p j n", j=2),
                                mybir.ActivationFunctionType.Exp,
                                bias=ebias[:], scale=ATT_SCALE)
                    return E_t

                def v_proj():
                    wv = wv_pool.tile([P, CO, HL * D], F16, tag="wv")
                    nc.sync.dma_start(wv[:], wv_pk[:])
                    for mo in range(MO):
                        pv = ps_proj.tile([P, NH], F32, tag="pp",
                                          name=f"pv{mo}")
                        for co in range(CO):
                            nc.tensor.matmul(
                                pv[:], xT[:, co, mo * P:(mo + 1) * P],
                                wv[:, co, :],
                                start=(co == 0), stop=(co == CO - 1))
                        nc.vector.tensor_scalar_mul(
                            v_sb[:, mo, :, 0:D],
                            pv[:].rearrange("p (h d) -> p h d", d=D),
                            mask_sb[:, mo:mo + 1])
                    for mo in range(MO):
                        nc.vector.tensor_scalar_mul(
                            v_sb[:, mo, :, D], onesH[:], mask_sb[:, mo:mo + 1])

                o_un = {}
                norm_state = {}

                def norm_qh(quad, qh):
                    # normalize the quad's q-half: fast 1/den on DVE,
                    # ones-row PE broadcast, multiply into o^T
                    den_q = norm_state[quad]
                    if qh == 0:
                        norm_state["rcp"] = sm_pool.tile(
                            [97, N], F32, tag="sm", name=f"rcp{quad}")
                        norm_state["rcph"] = rb_pool.tile(
                            [97, N], F16, tag="rb", name=f"rcph{quad}")
                    rcp_q, rcp_h = norm_state["rcp"], norm_state["rcph"]
                    sl = slice(qh * NH, (qh + 1) * NH)
                    nc.vector.reciprocal_approx_fast(
                        rcp_q[:, sl], den_q[:, sl])
                    nc.vector.tensor_copy(rcp_h[:, sl], rcp_q[:, sl])
                    for r in range(4):
                        h = 4 * quad + r
                        t, pb = h // 2, 64 * (h % 2)
                        pbc = ps_av.tile([P, NH], F32, tag="ps_av",
                                         name=f"pbc{h}_{qh}")
                        nc.tensor.matmul(
                            pbc[0:64, :], ones_q[32 * r:32 * r + 1, :],
                            rcp_h[32 * r:32 * r + 1, sl],
                            start=True, stop=True,
                            tile_position=(32 * r, 0))
                        nc.vector.tensor_mul(
                            o_sb[pb:pb + 64, t, sl],
                            o_un[h][:, sl], pbc[0:64, :])
                        if qh == 1:
                            o_un.pop(h)

                def av(t, E_t, mid=None):
                    # unnormalized out^T + denominator via the ones column
                    # (quad's head r at partition 32r: engine partition
                    # offsets must be multiples of 32)
                    if t % 2 == 0:
                        den_q = sm_pool.tile([97, N], F32, tag="sm",
                                             name=f"den{t // 2}")
                        nc.vector.memset(den_q[:], 1.0)
                        norm_state[t // 2] = den_q
                    den_q = norm_state[t // 2]
                    for j in range(2):
                        o_un[2 * t + j] = ou_pool.tile(
                            [64, N], F16, tag="ou", name=f"ou{2 * t + j}")
                    for qh in range(2):
                        for j in range(2):
                            h = 2 * t + j
                            r = h % 4
                            pav = ps_av.tile([P, NH], F32, tag="ps_av",
                                             name=f"pav{h}_{qh}")
                            for mo in range(MO):
                                nc.tensor.matmul(
                                    pav[:], v_sb[:, mo, h, :],
                                    E_t[:, mo, j, qh * NH:(qh + 1) * NH],
                                    start=(mo == 0), stop=(mo == MO - 1))
                            nc.vector.tensor_copy(
                                o_un[h][:, qh * NH:(qh + 1) * NH],
                                pav[0:D, :])
                            nc.vector.tensor_copy(
                                den_q[32 * r:32 * r + 1,
                                      qh * NH:(qh + 1) * NH],
                                pav[D:D + 1, :])
                        if qh == 0 and mid is not None:
                            mid()

                # skewed pipeline: attention of pair t overlaps k/q
                # projections of pair t+1 and S/exp of pair t+1.
                kq_proj(0, wk=wk0, wq=wq0)
                Es = {0: s_exp(0)}
                kq_proj(1)
                Es[1] = s_exp(1)
                kq_proj(2)
                v_proj()
                # norm chains are emitted with a lag so their PE broadcasts
                # never wait on the DVE reciprocal/cast pipeline
                for t in range(T):
                    if t + 2 < T:
                        Es[t + 2] = s_exp(t + 2)
                    if t + 3 < T:
                        kq_proj(t + 3)
                    if t == 1:  # stage w_out late, off the critical DMA path
                        for eo in range(EO):
                            nc.sync.dma_start(wo[:, eo, :], wo_pk[:, eo, :])
                    if t == 2:
                        norm_qh(0, 0)
                    av(t, Es.pop(t),
                       mid=(lambda: norm_qh(0, 1)) if t == 2 else None)
                norm_qh(1, 0)
                norm_qh(1, 1)

                # ---- partial output projection (host adds bias) ----
                for no in range(NO):
                    for ch in range(2):
                        pool = ps_proj if (no * 2 + ch) % 2 == 0 else ps_av
                        py = pool.tile([P, NH], F32,
                                       tag="pp" if pool is ps_proj
                                       else "ps_av",
                                       name=f"py{no}_{ch}")
                        for eo in range(EO):
                            nc.tensor.matmul(
                                py[:], o_sb[:, eo, no * P:(no + 1) * P],
                                wo[:, eo, ch * NH:(ch + 1) * NH],
                                start=(eo == 0), stop=(eo == EO - 1))
                        ysb = ysb_pool.tile([P, NH], F16, tag="ysb",
                                            name=f"ysb{no}_{ch}")
                        if no == NO - 1 and ch == 1:
                            nc.vector.tensor_copy(ysb[:], py[:])
                            nc.sync.dma_start(
                                y_t[:, no, ch * NH:ch * NH + 256],
                                ysb[:, 0:256])
                            nc.scalar.dma_start(
                                y_t[:, no, ch * NH + 256:(ch + 1) * NH],
                                ysb[:, 256:NH])
                        else:
                            nc.scalar.copy(ysb[:], py[:])
                            nc.sync.dma_start(
                                y_t[:, no, ch * NH:(ch + 1) * NH], ysb[:])

    nc.finalize()
    return nc


_NC_CACHE = None


def _get_nc():
    global _NC_CACHE
    if _NC_CACHE is None:
        _NC_CACHE = build_nc()
    return _NC_CACHE


def _make_in_maps(x, mask, w_qkv, w_out, b_out):
    x = np.asarray(x, dtype=np.float32)
    mask_f = np.asarray(mask).astype(np.float32)
    wqkv_h = np.asarray(w_qkv).astype(np.float16)
    wout_h = np.asarray(w_out).astype(np.float16)
    # w_qkv [C, 3HD]: q cols 0:C, k cols C:2C, v cols 2C:3C; head h at h*D
    wq4 = wqkv_h.reshape(CO, P, 3 * H * D)
    in_maps = []
    for i in range(N_CORES):
        b, hh = i // 2, i % 2
        e0 = hh * HL * D  # first e-col of this core's head block
        xbT = np.ascontiguousarray(x[b].T.astype(np.float16))
        wq_pk = np.ascontiguousarray(
            wq4[:, :, e0:e0 + HL * D].reshape(CO, P, T, P)
            .transpose(2, 1, 0, 3))
        wk_pk = np.ascontiguousarray(
            wq4[:, :, C + e0:C + e0 + HL * D].reshape(CO, P, T, P)
            .transpose(2, 1, 0, 3))
        wv_pk = np.ascontiguousarray(
            wq4[:, :, 2 * C + e0:2 * C + e0 + HL * D].transpose(1, 0, 2))
        wo_pk = np.ascontiguousarray(
            wout_h[e0:e0 + HL * D, :].reshape(EO, P, C).transpose(1, 0, 2))
        in_maps.append({"xbT": xbT, "maskb": mask_f[b], "wq_pk": wq_pk,
                        "wk_pk": wk_pk, "wv_pk": wv_pk, "wo_pk": wo_pk})
    return in_maps


def run_kernel(x, mask, w_qkv, w_out, b_out, trace=False):
    """Run on 8 cores; returns (full output [B,N,C], BassKernelResults)."""
    nc = _get_nc()
    in_maps = _make_in_maps(x, mask, w_qkv, w_out, b_out)
    res = run_bass_kernel_spmd(nc, in_maps, core_ids=list(range(N_CORES)),
                               trace=trace)
    bias = np.asarray(b_out, dtype=np.float32)
    out = np.empty((B, N, C), dtype=np.float32)
    for b in range(B):
        out[b] = (res.results[2 * b]["y"].astype(np.float32)
                  + res.results[2 * b + 1]["y"].astype(np.float32) + bias)
    return out, res


def kernel(x, mask, w_qkv, w_out, b_out):
    os.environ.setdefault("BASS_NEVER_TRACE", "1")
    out, _ = run_kernel(x, mask, w_qkv, w_out, b_out, trace=False)
    return out
